# revision 27
# baseline (speedup 1.0000x reference)
"""Trainium2 Bass kernel for a dense multi-head attention block.

Full (unsharded) contract: kernel(**inputs) -> np.ndarray [2, 2048, 1024].

Sharding: 8 cores = 2 (batch) x 4 (head-group of 4 heads).  Each core
computes Q/K/V projections for its 4 heads, RoPE, causal attention, and
a partial output (attn_group @ wo_rows).  The 4 partials per batch are
summed on the host (the tensor-parallel unshard).

v9: everything f16 on device (f32 PSUM accumulation), emitted as a
fine-grained interleave of two pipeline stages over 512-wide sequence
chunks so no engine ever stalls long enough to drop the PE clock:
stage1(sc) = Q/K/V projections + RoPE for chunk sc; stage2(qc=sc-1) =
causal attention for the previous chunk's queries.  Both stages share
one 8-bank PSUM pool (all tags single-buffered); emission alternates
~0.5us work units from each stage, so cross-engine waits (exp draining
scores PSUM, DVE draining projections) are always covered by
independent matmuls.  Scores arenas hold exp'd probs keyed [key, query]
per head; PV is computed transposed (V stationary, probs 512-wide
moving) giving attnT [head_dim, seq] directly (no transposes) plus
softmax denominators as PSUM row 64.  Denominators from all four heads
collect into one [128, 512] tile (rows 0/32/64/96) for a single DVE
reciprocal per chunk; normalization (PE ones-broadcast + DVE multiply)
and the wo projection are deferred one chunk so the reciprocal latency
hides under the next chunk's work.
"""

import os
import sys
import types

import numpy as np

B, S, D, H = 2, 2048, 1024, 16
HD = D // H          # 64
NHG = 4              # head-groups (tensor-parallel dim)
NH = 4               # heads per core
DHG = 256            # head dims per core
P = 128
N_CORES = 8
KT = D // P          # 8 contraction tiles for the projections
NST = S // P         # 16 sequence tiles
SC = 512             # sequence chunk (pipeline granularity)
NSC = S // SC        # 4 chunks

_CACHE = {}


def _install_trace_shim():
    """Make antenv.axon_hooks importable so bass_utils trace=True works."""
    if "antenv.axon_hooks" in sys.modules:
        return
    try:
        import trn_agent_boot.trn_boot as _tb
        hook = _tb._ntff_profile_via_ctypes("/opt/axon/libaxon_pjrt.so")
    except Exception:
        hook = None
    mod = types.ModuleType("antenv.axon_hooks")
    mod.get_axon_ntff_profile_hook = lambda: hook
    mod.set_axon_ntff_profile_hook = lambda h: None
    sys.modules["antenv.axon_hooks"] = mod


def _emit(tc, nc, ap, out_ap, mybir, dbg=None):
    from contextlib import ExitStack

    f32 = mybir.dt.float32
    f16 = mybir.dt.float16
    Exp = mybir.ActivationFunctionType.Exp

    with ExitStack() as ctx:
        consts = ctx.enter_context(tc.tile_pool(name="consts", bufs=1))

        wo_sb = consts.tile([P, 2, D], f16)
        pm_sb = consts.tile([P, P], f16)
        maskd_sb = consts.tile([P, P], f16)
        ones4 = consts.tile([P, 64], f16)
        nc.vector.memset(ones4, 1.0)

        qrot = consts.tile([P, 2, S], f16)
        krot = consts.tile([P, 2, S], f16)
        v65 = consts.tile([P, NST, NH, HD + 1], f16)
        nc.vector.memset(v65[:, :, :, HD:HD + 1], 1.0)
        den4 = consts.tile([P, SC], f32)
        nc.vector.memset(den4, 1.0)
        rcp4 = consts.tile([P, SC], f16)
        rcp_h3 = consts.tile([1, SC], f16)

        s1c = ctx.enter_context(tc.tile_pool(name="s1c", bufs=1))
        xT_sb = s1c.tile([P, KT, S], f16)
        xTr = ap["xT"].rearrange("(a p) s -> p a s", p=P)
        w_sb = {}
        for wn in ("wq", "wk", "wv"):
            w_sb[wn] = s1c.tile([P, KT, DHG], f16, name=f"w_{wn}", tag=f"w_{wn}")
        for a in range(KT):
            for wn in ("wq", "wk", "wv"):
                nc.scalar.dma_start(
                    out=w_sb[wn][:, a, :],
                    in_=ap[wn].rearrange("(a p) m -> p a m", p=P)[:, a, :])
        nc.scalar.dma_start(out=pm_sb, in_=ap["pm"])
        cs_sb = {}
        for cn in ("ccq", "ssq", "cck", "ssk"):
            cs_sb[cn] = s1c.tile([P, S], f16, name=f"cs_{cn}", tag=f"cs_{cn}")
            nc.scalar.dma_start(out=cs_sb[cn], in_=ap[cn])
        nc.scalar.dma_start(out=maskd_sb, in_=ap["maskd"])
        nc.scalar.dma_start(out=wo_sb, in_=ap["wo"].rearrange("(a p) m -> p a m", p=P))

        s1tmp = ctx.enter_context(tc.tile_pool(name="s1tmp", bufs=3))
        arena_p = ctx.enter_context(
            tc.tile_pool(name="arena", bufs=4 if dbg is not None else 2))
        attnT_p = ctx.enter_context(tc.tile_pool(name="attnT", bufs=2))
        smal = ctx.enter_context(tc.tile_pool(name="smal", bufs=4))
        obp = ctx.enter_context(tc.tile_pool(name="obp", bufs=4))
        ps = ctx.enter_context(tc.tile_pool(name="ps", bufs=1, space="PSUM"))

        qk_specs = (("wq", "ccq", "ssq", qrot), ("wk", "cck", "ssk", krot))
        prs = {}
        arenas = {}
        attnTs = {}

        def s1_units(sc):
            units = []
            ssl = slice(sc * SC, (sc + 1) * SC)

            def u_dma():
                if sc == 0:
                    for a in range(KT):
                        nc.sync.dma_start(out=xT_sb[:, a, ssl],
                                          in_=xTr[:, a, ssl])
                if sc + 1 < NSC:
                    nsl = slice((sc + 1) * SC, (sc + 2) * SC)
                    for a in range(KT):
                        nc.sync.dma_start(out=xT_sb[:, a, nsl],
                                          in_=xTr[:, a, nsl])
            units.append((u_dma, 0.05))

            for wn, ccn, ssn, rot in qk_specs:
                for a in range(KT):
                    def u_mm(wn=wn, a=a):
                        if a == 0:
                            for m in range(2):
                                prs[(wn, m)] = ps.tile(
                                    [P, SC], f32, tag=f"pr{m}", bufs=1,
                                    name=f"pr_{wn}_{m}_{sc}")
                        for m in range(2):
                            nc.tensor.matmul(
                                prs[(wn, m)],
                                w_sb[wn][:, a, m * P:(m + 1) * P],
                                xT_sb[:, a, ssl],
                                start=(a == 0), stop=(a == KT - 1))
                    units.append((u_mm, 0.45))
                for m in range(2):
                    def u_rope(wn=wn, m=m, ccn=ccn, ssn=ssn, rot=rot):
                        ev = s1tmp.tile([P, SC], f16, tag="ev",
                                        name=f"ev{wn}{m}_{sc}")
                        nc.vector.tensor_copy(ev, prs[(wn, m)])
                        sw = ps.tile([P, SC], f32, tag="sw", bufs=1,
                                     name=f"sw{wn}{m}_{sc}")
                        nc.tensor.matmul(sw, pm_sb, ev, start=True, stop=True)
                        t1 = s1tmp.tile([P, SC], f16, tag="t1",
                                        name=f"t1{wn}{m}_{sc}")
                        nc.gpsimd.tensor_mul(t1, ev, cs_sb[ccn][:, ssl])
                        t2 = s1tmp.tile([P, SC], f16, tag="t2",
                                        name=f"t2{wn}{m}_{sc}")
                        nc.vector.tensor_mul(t2, sw, cs_sb[ssn][:, ssl])
                        nc.vector.tensor_add(rot[:, m, ssl], t1, t2)
                    units.append((u_rope, 0.3))
            for st in range(SC // P):
                def u_v(st=st):
                    stg = sc * (SC // P) + st
                    vp = ps.tile([P, DHG], f32, tag="vp", bufs=1,
                                 name=f"vp{stg}")
                    for a in range(KT):
                        nc.tensor.matmul(
                            vp,
                            xT_sb[:, a, stg * P:(stg + 1) * P],
                            w_sb["wv"][:, a, :],
                            start=(a == 0), stop=(a == KT - 1))
                    nc.vector.tensor_copy(v65[:, stg, :, 0:HD],
                                          vp.rearrange("p (h d) -> p h d",
                                                       h=NH))
                units.append((u_v, 1.0))
            return units

        def norm_units(qc):
            units = []
            for t in range(2):
                def u_norm(t=t, qc=qc):
                    attnT = attnTs[qc]
                    bcp = ps.tile([P, SC], f32, tag="bcwo", bufs=1,
                                  name=f"bc{t}_{qc}")
                    for par in range(2):
                        h = 2 * t + par
                        if h == 3:
                            lhs, rhs = ones4[0:1, :], rcp_h3
                        else:
                            lhs = ones4[32 * h:32 * h + 1, :]
                            rhs = rcp4[32 * h:32 * h + 1, :]
                        nc.tensor.matmul(bcp[64 * par:64 * par + 64, :],
                                         lhs, rhs, start=True, stop=True)
                    for par in range(2):
                        h = 2 * t + par
                        sl = slice(64 * par, 64 * par + 64)
                        nc.vector.tensor_mul(attnT[sl, h // 2, :],
                                             attnT[sl, h // 2, :],
                                             bcp[sl, :])
                units.append((u_norm, 0.5))
            return units

        def wo_units(qc):
            units = []
            for ml in range(SC // P):
                for nn in range(2):
                    def u_wo(ml=ml, nn=nn, qc=qc):
                        attnT = attnTs[qc]
                        m = qc * (SC // P) + ml
                        wo_ps = ps.tile([P, SC], f32, tag="bcwo", bufs=1,
                                        name=f"wo{m}_{nn}")
                        for kd in range(2):
                            nc.tensor.matmul(
                                wo_ps,
                                attnT[:, kd, ml * P:(ml + 1) * P],
                                wo_sb[:, kd, nn * 512:(nn + 1) * 512],
                                start=(kd == 0), stop=(kd == 1))
                        ob = obp.tile([P, 512], f16, tag="ob",
                                      name=f"ob{m}_{nn}")
                        if nn == 0:
                            nc.vector.tensor_copy(ob, wo_ps)
                        else:
                            nc.scalar.copy(ob, wo_ps)
                        nc.sync.dma_start(
                            out=out_ap[m * P:(m + 1) * P,
                                       nn * 512:(nn + 1) * 512],
                            in_=ob)
                    units.append((u_wo, 0.5))
            return units

        def s2_units(qc, pend):
            nt = 4 * qc + 4
            units = []
            tail = (list(norm_units(pend)) + list(wo_units(pend))
                    if pend is not None else [])
            # tail of the previous chunk is woven in after each head
            tail_chunks = [tail[0:2], tail[2:6], tail[6:10], tail[10:14],
                           tail[14:]]
            for h in range(NH):
                t, po = h // 2, 64 * (h % 2)
                for ip in range(0, nt, 2):
                    def u_piece(h=h, ip=ip, t=t, po=po):
                        if ip == 0:
                            if h == 0:
                                attnTs[qc] = attnT_p.tile(
                                    [P, 2, SC], f16, tag="at", name=f"at{qc}")
                            arenas[h] = arena_p.tile(
                                [P, NST, SC], f16, tag="ar", name=f"ar{h}_{qc}")
                        i0, i1 = ip, ip + 1
                        start = max(i0 * P - qc * SC, 0)
                        scr = ps.tile([P, 2, SC], f32, tag="scr", bufs=1,
                                      name=f"scr{h}_{qc}_{ip}")
                        for j, i in ((0, i0), (1, i1)):
                            nc.tensor.matmul(
                                scr[:, j, start:SC],
                                krot[po:po + 64, t, i * P:(i + 1) * P],
                                qrot[po:po + 64, t,
                                     qc * SC + start:(qc + 1) * SC],
                                start=True, stop=True)
                        for j, i in ((0, i0), (1, i1)):
                            if i >= 4 * qc:   # diagonal tile: causal mask
                                dcol = i * P - qc * SC
                                nc.vector.tensor_add(
                                    scr[:, j, dcol:dcol + P],
                                    scr[:, j, dcol:dcol + P], maskd_sb)
                        nc.scalar.activation(
                            arenas[h][:, i0:i0 + 2, start:SC],
                            scr[:, :, start:SC], Exp)
                    units.append((u_piece, 0.5))

                def u_pvt(h=h, qc=qc, nt=nt):
                    pv = ps.tile([P, SC], f32, tag="pv", bufs=1,
                                 name=f"pv{h}_{qc}")
                    for i in range(nt):
                        scol = max(i * P - qc * SC, 0)
                        nc.tensor.matmul(
                            pv[0:65, scol:SC],
                            v65[:, i, h, :],
                            arenas[h][:, i, scol:SC],
                            start=(i == 0), stop=(i == nt - 1))
                    if dbg is not None and qc == 3 and h == 0:
                        pvd = smal.tile([P, SC], f32, tag="pvd", name="pvd")
                        nc.vector.tensor_copy(pvd[0:65, :], pv[0:65, :])
                        nc.sync.dma_start(out=dbg["pv30"], in_=pvd[0:65, :])
                    nc.vector.tensor_copy(
                        attnTs[qc][64 * (h % 2):64 * (h % 2) + 64,
                                   h // 2, :],
                        pv[0:64, :])
                    nc.scalar.copy(den4[32 * h:32 * h + 1, :], pv[64:65, :])
                    last = qc == NSC - 1
                    if h == 1 and last:
                        with nc.allow_low_precision(reason="f16 denoms"):
                            nc.vector.reciprocal(rcp4[0:64, :], den4[0:64, :])
                    if h == 3:
                        with nc.allow_low_precision(reason="f16 denoms"):
                            if last:
                                nc.vector.reciprocal(rcp4[64:128, :],
                                                     den4[64:128, :])
                            else:
                                nc.vector.reciprocal(rcp4, den4)
                        nc.vector.tensor_copy(rcp_h3, rcp4[96:97, :])
                units.append((u_pvt, 0.22 * nt))
                units.extend(tail_chunks[h])
            units.extend(tail_chunks[4])
            return units

        def merge(ua, ub):
            ca = sum(c for _, c in ua) or 1e-9
            cb = sum(c for _, c in ub) or 1e-9
            out = []
            ia = ib = 0
            sa = sb = 0.0
            while ia < len(ua) or ib < len(ub):
                if ib >= len(ub) or (ia < len(ua) and sa * cb <= sb * ca):
                    out.append(ua[ia]); sa += ua[ia][1]; ia += 1
                else:
                    out.append(ub[ib]); sb += ub[ib][1]; ib += 1
            return out

        for u, _ in s1_units(0):
            u()
        pend = None
        for sc in range(1, NSC):
            for u, _ in merge(s1_units(sc), s2_units(sc - 1, pend)):
                u()
            pend = sc - 1
        for u, _ in s2_units(NSC - 1, pend):
            u()
        for u, _ in norm_units(NSC - 1) + wo_units(NSC - 1):
            u()

        if dbg is not None:
            nc.sync.dma_start(out=dbg["qrot"], in_=qrot)
            nc.sync.dma_start(out=dbg["krot"], in_=krot)
            nc.sync.dma_start(out=dbg["v65"], in_=v65)
            nc.sync.dma_start(out=dbg["ar0"], in_=arenas[0])
            nc.sync.dma_start(out=dbg["at3"], in_=attnTs[3])


def _build_program(debug=False):
    import concourse.tile as tile
    import concourse.mybir as mybir
    from concourse import bacc

    f32 = mybir.dt.float32
    f16 = mybir.dt.float16

    nc = bacc.Bacc("TRN2", target_bir_lowering=False, debug=False,
                   num_devices=N_CORES)
    ap = {}

    def inp(name, shape, dt):
        ap[name] = nc.dram_tensor(name, shape, dt, kind="ExternalInput").ap()

    inp("xT", [D, S], f16)
    inp("wq", [D, DHG], f16)
    inp("wk", [D, DHG], f16)
    inp("wv", [D, DHG], f16)
    inp("wo", [DHG, D], f16)
    inp("ccq", [P, S], f16)
    inp("ssq", [P, S], f16)
    inp("cck", [P, S], f16)
    inp("ssk", [P, S], f16)
    inp("maskd", [P, P], f16)
    inp("pm", [P, P], f16)
    out_ap = nc.dram_tensor("out", [S, D], f16, kind="ExternalOutput").ap()
    dbg = None
    if debug:
        dbg = {
            "qrot": nc.dram_tensor("dbg_qrot", [P, 2, S], f16, kind="ExternalOutput").ap(),
            "krot": nc.dram_tensor("dbg_krot", [P, 2, S], f16, kind="ExternalOutput").ap(),
            "v65": nc.dram_tensor("dbg_v65", [P, NST, NH, HD + 1], f16, kind="ExternalOutput").ap(),
            "ar0": nc.dram_tensor("dbg_ar0", [P, NST, SC], f16, kind="ExternalOutput").ap(),
            "at3": nc.dram_tensor("dbg_at3", [P, 2, SC], f16, kind="ExternalOutput").ap(),
            "pv30": nc.dram_tensor("dbg_pv30", [65, SC], f32, kind="ExternalOutput").ap(),
        }

    with tile.TileContext(nc) as tc:
        _emit(tc, nc, ap, out_ap, mybir, dbg=dbg)
    nc.compile()
    return nc


def _host_prep(x, wq, wk, wv, wo, freqs_cos, freqs_sin, mask):
    """Build the 8 per-core input maps."""
    perm = []
    for h in range(NH):
        perm += [HD * h + 2 * j for j in range(HD // 2)]
        perm += [HD * h + 2 * j + 1 for j in range(HD // 2)]
    perm = np.asarray(perm)

    cosT = np.ascontiguousarray(freqs_cos.T).astype(np.float32)   # [32, S]
    sinT = np.ascontiguousarray(freqs_sin.T).astype(np.float32)
    CC = np.tile(cosT, (4, 1))                                    # [128, S]
    SS = np.tile(np.vstack([-sinT, sinT]), (2, 1))                # [128, S]
    ccq, ssq = (CC * 0.125).astype(np.float16), (SS * 0.125).astype(np.float16)
    cck, ssk = CC.astype(np.float16), SS.astype(np.float16)

    swap = np.zeros((P, P), dtype=np.float16)
    for g in range(2):
        for j in range(32):
            swap[64 * g + 32 + j, 64 * g + j] = 1.0
            swap[64 * g + j, 64 * g + 32 + j] = 1.0

    m2 = mask[0, 0]
    # transposed to [key, query] to match the scores arena layout
    maskd = np.clip(m2[0:P, 0:P].T, -30000.0, 30000.0).astype(np.float16)

    xT = [np.ascontiguousarray(x[b].T).astype(np.float16) for b in range(B)]

    in_maps = []
    for c in range(N_CORES):
        b, hg = c // NHG, c % NHG
        cols = hg * DHG + np.arange(DHG)
        in_maps.append({
            "xT": xT[b],
            "wq": np.ascontiguousarray(wq[:, hg * DHG + perm]).astype(np.float16),
            "wk": np.ascontiguousarray(wk[:, hg * DHG + perm]).astype(np.float16),
            "wv": np.ascontiguousarray(wv[:, cols]).astype(np.float16),
            "wo": np.ascontiguousarray(wo[cols, :]).astype(np.float16),
            "ccq": ccq, "ssq": ssq, "cck": cck, "ssk": ssk,
            "maskd": maskd, "pm": swap,
        })
    return in_maps


def kernel(x, wq, wk, wv, wo, freqs_cos, freqs_sin, mask, start_pos=0, **_):
    import concourse.bass_utils as bass_utils

    x = np.asarray(x, dtype=np.float32)
    wq = np.asarray(wq, dtype=np.float32)
    wk = np.asarray(wk, dtype=np.float32)
    wv = np.asarray(wv, dtype=np.float32)
    wo = np.asarray(wo, dtype=np.float32)
    freqs_cos = np.asarray(freqs_cos, dtype=np.float32)
    freqs_sin = np.asarray(freqs_sin, dtype=np.float32)
    mask = np.asarray(mask, dtype=np.float32)

    trace = bool(int(os.environ.get("BASS_KERNEL_TRACE", "0")))
    if trace:
        _install_trace_shim()
        import concourse.bass_utils as bu
        bu.upload_artifacts = lambda tmpdir: "(upload skipped)"

    if "nc" not in _CACHE:
        _CACHE["nc"] = _build_program()
    nc = _CACHE["nc"]

    in_maps = _host_prep(x, wq, wk, wv, wo, freqs_cos, freqs_sin, mask)
    kwargs = {}
    if trace:
        kwargs = dict(trace=True, trace_cores=[0],
                      tmpdir=os.environ.get("BASS_KERNEL_TRACE_DIR", None))
    res = None
    last_exc = None
    for attempt in range(5):
        try:
            res = bass_utils.run_bass_kernel_spmd(
                nc, in_maps, core_ids=list(range(N_CORES)), **kwargs)
            break
        except Exception as e:  # transient NRT device errors recover on retry
            last_exc = e
            import time as _time
            _time.sleep(12)
    if res is None:
        raise last_exc
    _CACHE["last_result"] = res

    out = np.zeros((B, S, D), dtype=np.float32)
    for c in range(N_CORES):
        out[c // NHG] += res.results[c]["out"].astype(np.float32)
    return out


# revision 28
# speedup vs baseline: 1.2914x; 1.2914x over previous
"""Trainium2 Bass kernel for a dense multi-head attention block.

Full (unsharded) contract: kernel(**inputs) -> np.ndarray [2, 2048, 1024].

Sharding: 8 cores = 2 (batch) x 4 (head-group of 4 heads).  Each core
computes Q/K/V projections for its 4 heads, RoPE, causal attention, and
a partial output (attn_group @ wo_rows).  The 4 partials per batch are
summed on the host (the tensor-parallel unshard).

v2 layout: everything f16 on device (f32 PSUM accumulation).  The kernel
is a single software-pipelined loop over 512-wide sequence chunks:
stage1(sc) computes Q/K/V projections + RoPE for chunk sc, then
stage2(qc=sc) runs causal attention for the 512 queries of that chunk
against all keys <= chunk end.  Scores matmuls are row-packed two heads
at a time (K=64 each, tile_position auto from base partition) and flow
through 1024-wide PSUM pieces drained by ACT exp.  PV is computed
transposed (V as stationary, probs as 512-wide moving operand), which
yields the attention output directly in [head_dim, seq] layout for the
wo projection (no transposes) plus softmax denominators as PSUM row 64.
Normalization: DVE reciprocal of the denominator row, PE ones-matmul
broadcast to 64 partitions, DVE multiply into the attnT tile.
"""

import os
import sys
import types

import numpy as np

B, S, D, H = 2, 2048, 1024, 16
HD = D // H          # 64
NHG = 4              # head-groups (tensor-parallel dim)
NH = 4               # heads per core
DHG = 256            # head dims per core
P = 128
N_CORES = 8
KT = D // P          # 8 contraction tiles for the projections
NST = S // P         # 16 sequence tiles
SC = 512             # sequence chunk (pipeline granularity)
NSC = S // SC        # 4 chunks

_CACHE = {}


def _install_trace_shim():
    """Make antenv.axon_hooks importable so bass_utils trace=True works."""
    if "antenv.axon_hooks" in sys.modules:
        return
    try:
        import trn_agent_boot.trn_boot as _tb
        hook = _tb._ntff_profile_via_ctypes("/opt/axon/libaxon_pjrt.so")
    except Exception:
        hook = None
    mod = types.ModuleType("antenv.axon_hooks")
    mod.get_axon_ntff_profile_hook = lambda: hook
    mod.set_axon_ntff_profile_hook = lambda h: None
    sys.modules["antenv.axon_hooks"] = mod


def _emit(tc, nc, ap, out_ap, mybir, dbg=None):
    from contextlib import ExitStack

    f32 = mybir.dt.float32
    f16 = mybir.dt.float16
    f32r = mybir.dt.float32r
    Exp = mybir.ActivationFunctionType.Exp
    Ln = mybir.ActivationFunctionType.Ln

    with ExitStack() as ctx:
        consts = ctx.enter_context(tc.tile_pool(name="consts", bufs=1))

        wo_sb = consts.tile([P, 2, D], f16)
        pm_sb = consts.tile([P, P], f16)
        maskd_sb = consts.tile([P, P], f16)
        ones4 = consts.tile([P, 64], f16)
        nc.vector.memset(ones4, 1.0)

        qrot = consts.tile([P, 2, S], f16)
        krot = consts.tile([P, 2, S], f16)
        v65 = consts.tile([P, NST, NH, HD + 1], f16)
        nc.vector.memset(v65[:, :, :, HD:HD + 1], 1.0)
        den4 = consts.tile([P, SC], f32)
        nc.vector.memset(den4, 1.0)
        rcp4 = consts.tile([P, SC], f16)
        rcp_h3 = consts.tile([1, SC], f16)

        s1c = ctx.enter_context(tc.tile_pool(name="s1c", bufs=1))
        xT_sb = s1c.tile([P, KT, S], f16)
        xTr = ap["xT"].rearrange("(a p) s -> p a s", p=P)
        w_sb = {}
        for wn in ("wq", "wk", "wv"):
            w_sb[wn] = s1c.tile([P, KT, DHG], f16, name=f"w_{wn}", tag=f"w_{wn}")
        for a in range(KT):
            for wn in ("wq", "wk", "wv"):
                nc.scalar.dma_start(
                    out=w_sb[wn][:, a, :],
                    in_=ap[wn].rearrange("(a p) m -> p a m", p=P)[:, a, :])
        nc.scalar.dma_start(out=pm_sb, in_=ap["pm"])
        cs_sb = {}
        for cn in ("ccq", "ssq", "cck", "ssk"):
            cs_sb[cn] = s1c.tile([P, S], f16, name=f"cs_{cn}", tag=f"cs_{cn}")
            nc.scalar.dma_start(out=cs_sb[cn], in_=ap[cn])
        nc.scalar.dma_start(out=maskd_sb, in_=ap["maskd"])
        nc.scalar.dma_start(out=wo_sb, in_=ap["wo"].rearrange("(a p) m -> p a m", p=P))

        s1tmp = ctx.enter_context(tc.tile_pool(name="s1tmp", bufs=3))
        arena_p = ctx.enter_context(tc.tile_pool(name="arena", bufs=1))
        attnT_p = ctx.enter_context(tc.tile_pool(name="attnT", bufs=2))
        smal = ctx.enter_context(tc.tile_pool(name="smal", bufs=4))
        obp = ctx.enter_context(tc.tile_pool(name="obp", bufs=4))

        qk_specs = (("wq", "ccq", "ssq", qrot), ("wk", "cck", "ssk", krot))

        def stage1(sc, s1ps):
            ssl = slice(sc * SC, (sc + 1) * SC)
            if sc == 0:
                for a in range(KT):
                    nc.sync.dma_start(out=xT_sb[:, a, ssl], in_=xTr[:, a, ssl])
            if sc + 1 < NSC:
                nsl = slice((sc + 1) * SC, (sc + 2) * SC)
                for a in range(KT):
                    nc.sync.dma_start(out=xT_sb[:, a, nsl], in_=xTr[:, a, nsl])
            prs = {}
            for wn, ccn, ssn, rot in qk_specs:
                for m in range(2):
                    prs[(wn, m)] = s1ps.tile([P, SC], f32, tag=f"pr{wn}{m}",
                                             name=f"pr_{wn}_{m}", bufs=1)
            for a in range(KT):
                for wn, ccn, ssn, rot in qk_specs:
                    for m in range(2):
                        nc.tensor.matmul(
                            prs[(wn, m)],
                            w_sb[wn][:, a, m * P:(m + 1) * P],
                            xT_sb[:, a, ssl],
                            start=(a == 0), stop=(a == KT - 1))
            evs = {}
            for wn, ccn, ssn, rot in qk_specs:
                for m in range(2):
                    ev = s1tmp.tile([P, SC], f16, tag="ev", name=f"ev{wn}{m}")
                    nc.vector.tensor_copy(ev, prs[(wn, m)])
                    evs[(wn, m)] = ev
            # V in natural [seq, dim] layout (xT tiles as stationary operand)
            for st in range(SC // P):
                stg = sc * (SC // P) + st
                vp = s1ps.tile([P, DHG], f32, tag="vp", bufs=2, name=f"vp{stg}")
                for a in range(KT):
                    nc.tensor.matmul(
                        vp,
                        xT_sb[:, a, stg * P:(stg + 1) * P],
                        w_sb["wv"][:, a, :],
                        start=(a == 0), stop=(a == KT - 1))
                nc.vector.tensor_copy(v65[:, stg, :, 0:HD],
                                      vp.rearrange("p (h d) -> p h d", h=NH))
            # RoPE: pair-swap via PE, muls on gpsimd/vector
            for wn, ccn, ssn, rot in qk_specs:
                for m in range(2):
                    ev = evs[(wn, m)]
                    sw = s1ps.tile([P, SC], f32, tag="sw", bufs=2,
                                   name=f"sw{wn}{m}")
                    nc.tensor.matmul(sw, pm_sb, ev, start=True, stop=True)
                    t1 = s1tmp.tile([P, SC], f16, tag="t1", name=f"t1{wn}{m}")
                    nc.gpsimd.tensor_mul(t1, ev, cs_sb[ccn][:, ssl])
                    t2 = s1tmp.tile([P, SC], f16, tag="t2", name=f"t2{wn}{m}")
                    nc.vector.tensor_mul(t2, sw, cs_sb[ssn][:, ssl])
                    nc.vector.tensor_add(rot[:, m, ssl], t1, t2)

        arenas = {}
        attnTs = {}
        rcps = {}

        def emit_pieces(qc, t, s2ps):
            nt = 4 * qc + 4
            for h in (2 * t, 2 * t + 1):
                arenas[h] = arena_p.tile([P, NST, SC], f16, tag=f"ar{h}",
                                         name=f"ar{h}_{qc}")
            for ip in range(0, nt, 2):
                i0, i1 = ip, ip + 1
                start = max(i0 * P - qc * SC, 0)
                scrs = {}
                for h in (2 * t, 2 * t + 1):
                    scrs[h] = s2ps.tile([P, 2, SC], f32, tag=f"scr{h % 2}",
                                        bufs=1, name=f"scr{h}_{qc}_{ip}")
                for j, i in ((0, i0), (1, i1)):
                    for h in (2 * t, 2 * t + 1):
                        po = 64 * (h % 2)
                        nc.tensor.matmul(
                            scrs[h][:, j, start:SC],
                            krot[po:po + 64, t, i * P:(i + 1) * P],
                            qrot[po:po + 64, t,
                                 qc * SC + start:(qc + 1) * SC],
                            start=True, stop=True)
                for j, i in ((0, i0), (1, i1)):
                    if i >= 4 * qc:       # diagonal tile: causal mask
                        dcol = i * P - qc * SC
                        for h in (2 * t, 2 * t + 1):
                            nc.vector.tensor_add(
                                scrs[h][:, j, dcol:dcol + P],
                                scrs[h][:, j, dcol:dcol + P], maskd_sb)
                for h in (2 * t, 2 * t + 1):
                    nc.scalar.activation(
                        arenas[h][:, i0:i0 + 2, start:SC],
                        scrs[h][:, :, start:SC], Exp)

        def emit_pv(qc, s2ps, split=False):
            # transposed PV per head; unnormalized output + denominators
            # staged to SBUF so the PSUM pool can recycle; reciprocal of all
            # four heads in one DVE op (free-dim bound, partitions parallel)
            nt = 4 * qc + 4
            attnT = attnT_p.tile([P, 2, SC], f16, tag="at", name=f"at{qc}")
            attnTs[qc] = attnT
            for t in range(2):
                for h in (2 * t, 2 * t + 1):
                    pv = s2ps.tile([P, SC], f32, tag="pv", bufs=2,
                                   name=f"pv{h}_{qc}")
                    for i in range(nt):
                        scol = max(i * P - qc * SC, 0)
                        nc.tensor.matmul(
                            pv[0:65, scol:SC],
                            v65[:, i, h, :],
                            arenas[h][:, i, scol:SC],
                            start=(i == 0), stop=(i == nt - 1))
                    if dbg is not None and qc == 3 and h == 0:
                        pvd = smal.tile([P, SC], f32, tag="pvd", name="pvd")
                        nc.vector.tensor_copy(pvd[0:65, :], pv[0:65, :])
                        nc.sync.dma_start(out=dbg["pv30"], in_=pvd[0:65, :])
                    nc.vector.tensor_copy(
                        attnT[64 * (h % 2):64 * (h % 2) + 64, h // 2, :],
                        pv[0:64, :])
                    nc.scalar.copy(den4[32 * h:32 * h + 1, :], pv[64:65, :])
                if t == 0 and split:
                    with nc.allow_low_precision(reason="f16 softmax denoms"):
                        nc.vector.reciprocal(rcp4[0:64, :], den4[0:64, :])
            if split:
                with nc.allow_low_precision(reason="f16 softmax denoms"):
                    nc.vector.reciprocal(rcp4[64:128, :], den4[64:128, :])
            else:
                with nc.allow_low_precision(reason="f16 softmax denominators"):
                    nc.vector.reciprocal(rcp4, den4)
            nc.vector.tensor_copy(rcp_h3, rcp4[96:97, :])
            rcps[qc] = (rcp4, rcp_h3)

        def emit_norm(qc, s2ps):
            # broadcast rcp rows across 64 partitions via K=1 matmuls, then
            # normalize attnT in place (runs a block after emit_pv, so the
            # reciprocal latency is hidden behind the next stage1)
            attnT = attnTs[qc]
            rcp4, rcp_h3 = rcps[qc]
            for t in range(2):
                bcp = s2ps.tile([P, SC], f32, tag="bcwo", bufs=2,
                                name=f"bc{t}_{qc}")
                for par in range(2):
                    h = 2 * t + par
                    if h == 3:
                        lhs, rhs = ones4[0:1, :], rcp_h3
                    else:
                        lhs = ones4[32 * h:32 * h + 1, :]
                        rhs = rcp4[32 * h:32 * h + 1, :]
                    nc.tensor.matmul(bcp[64 * par:64 * par + 64, :],
                                     lhs, rhs, start=True, stop=True)
                for par in range(2):
                    h = 2 * t + par
                    sl = slice(64 * par, 64 * par + 64)
                    nc.vector.tensor_mul(attnT[sl, h // 2, :],
                                         attnT[sl, h // 2, :], bcp[sl, :])

        def emit_wo(qc, s2ps):
            attnT = attnTs[qc]
            for ml in range(SC // P):
                m = qc * (SC // P) + ml
                for nn in range(2):
                    wo_ps = s2ps.tile([P, SC], f32, tag="bcwo", bufs=2,
                                      name=f"wo{m}_{nn}")
                    for kd in range(2):
                        nc.tensor.matmul(
                            wo_ps,
                            attnT[:, kd, ml * P:(ml + 1) * P],
                            wo_sb[:, kd, nn * 512:(nn + 1) * 512],
                            start=(kd == 0), stop=(kd == 1))
                    ob = obp.tile([P, 512], f16, tag="ob", name=f"ob{m}_{nn}")
                    if nn == 0:
                        nc.vector.tensor_copy(ob, wo_ps)
                    else:
                        nc.scalar.copy(ob, wo_ps)
                    nc.sync.dma_start(
                        out=out_ap[m * P:(m + 1) * P, nn * 512:(nn + 1) * 512],
                        in_=ob)

        def stage2(qc, s2ps, pend):
            emit_pieces(qc, 0, s2ps)
            if pend is not None:
                emit_norm(pend, s2ps)
            emit_pieces(qc, 1, s2ps)
            if pend is not None:
                emit_wo(pend, s2ps)
            emit_pv(qc, s2ps, split=(qc == NSC - 1))

        with tc.tile_pool(name="s1ps0", bufs=1, space="PSUM") as s1ps:
            stage1(0, s1ps)
        pend = None
        for sc in range(1, NSC):
            with tc.tile_pool(name=f"s1ps{sc}", bufs=1, space="PSUM") as s1ps:
                stage1(sc, s1ps)
            with tc.tile_pool(name=f"s2ps{sc - 1}", bufs=1, space="PSUM") as s2ps:
                stage2(sc - 1, s2ps, pend)
                pend = sc - 1
        with tc.tile_pool(name=f"s2ps{NSC - 1}", bufs=1, space="PSUM") as s2ps:
            stage2(NSC - 1, s2ps, pend)
            emit_norm(NSC - 1, s2ps)
            emit_wo(NSC - 1, s2ps)

        if dbg is not None:
            nc.sync.dma_start(out=dbg["qrot"], in_=qrot)
            nc.sync.dma_start(out=dbg["krot"], in_=krot)
            nc.sync.dma_start(out=dbg["v65"], in_=v65)
            nc.sync.dma_start(out=dbg["ar0"], in_=arenas[0])
            nc.sync.dma_start(out=dbg["at3"], in_=attnTs[3])


def _build_program(debug=False):
    import concourse.tile as tile
    import concourse.mybir as mybir
    from concourse import bacc

    f32 = mybir.dt.float32
    f16 = mybir.dt.float16

    nc = bacc.Bacc("TRN2", target_bir_lowering=False, debug=False,
                   num_devices=N_CORES)
    ap = {}

    def inp(name, shape, dt):
        ap[name] = nc.dram_tensor(name, shape, dt, kind="ExternalInput").ap()

    inp("xT", [D, S], f16)
    inp("wq", [D, DHG], f16)
    inp("wk", [D, DHG], f16)
    inp("wv", [D, DHG], f16)
    inp("wo", [DHG, D], f16)
    inp("ccq", [P, S], f16)
    inp("ssq", [P, S], f16)
    inp("cck", [P, S], f16)
    inp("ssk", [P, S], f16)
    inp("maskd", [P, P], f16)
    inp("pm", [P, P], f16)
    out_ap = nc.dram_tensor("out", [S, D], f16, kind="ExternalOutput").ap()
    dbg = None
    if debug:
        dbg = {
            "qrot": nc.dram_tensor("dbg_qrot", [P, 2, S], f16, kind="ExternalOutput").ap(),
            "krot": nc.dram_tensor("dbg_krot", [P, 2, S], f16, kind="ExternalOutput").ap(),
            "v65": nc.dram_tensor("dbg_v65", [P, NST, NH, HD + 1], f16, kind="ExternalOutput").ap(),
            "ar0": nc.dram_tensor("dbg_ar0", [P, NST, SC], f16, kind="ExternalOutput").ap(),
            "at3": nc.dram_tensor("dbg_at3", [P, 2, SC], f16, kind="ExternalOutput").ap(),
            "pv30": nc.dram_tensor("dbg_pv30", [65, SC], f32, kind="ExternalOutput").ap(),
        }

    with tile.TileContext(nc) as tc:
        _emit(tc, nc, ap, out_ap, mybir, dbg=dbg)
    nc.compile()
    return nc


def _host_prep(x, wq, wk, wv, wo, freqs_cos, freqs_sin, mask):
    """Build the 8 per-core input maps."""
    perm = []
    for h in range(NH):
        perm += [HD * h + 2 * j for j in range(HD // 2)]
        perm += [HD * h + 2 * j + 1 for j in range(HD // 2)]
    perm = np.asarray(perm)

    cosT = np.ascontiguousarray(freqs_cos.T).astype(np.float32)   # [32, S]
    sinT = np.ascontiguousarray(freqs_sin.T).astype(np.float32)
    CC = np.tile(cosT, (4, 1))                                    # [128, S]
    SS = np.tile(np.vstack([-sinT, sinT]), (2, 1))                # [128, S]
    ccq, ssq = (CC * 0.125).astype(np.float16), (SS * 0.125).astype(np.float16)
    cck, ssk = CC.astype(np.float16), SS.astype(np.float16)

    swap = np.zeros((P, P), dtype=np.float16)
    for g in range(2):
        for j in range(32):
            swap[64 * g + 32 + j, 64 * g + j] = 1.0
            swap[64 * g + j, 64 * g + 32 + j] = 1.0

    m2 = mask[0, 0]
    # transposed to [key, query] to match the scores arena layout
    maskd = np.clip(m2[0:P, 0:P].T, -30000.0, 30000.0).astype(np.float16)

    xT = [np.ascontiguousarray(x[b].T).astype(np.float16) for b in range(B)]

    in_maps = []
    for c in range(N_CORES):
        b, hg = c // NHG, c % NHG
        cols = hg * DHG + np.arange(DHG)
        in_maps.append({
            "xT": xT[b],
            "wq": np.ascontiguousarray(wq[:, hg * DHG + perm]).astype(np.float16),
            "wk": np.ascontiguousarray(wk[:, hg * DHG + perm]).astype(np.float16),
            "wv": np.ascontiguousarray(wv[:, cols]).astype(np.float16),
            "wo": np.ascontiguousarray(wo[cols, :]).astype(np.float16),
            "ccq": ccq, "ssq": ssq, "cck": cck, "ssk": ssk,
            "maskd": maskd, "pm": swap,
        })
    return in_maps


def kernel(x, wq, wk, wv, wo, freqs_cos, freqs_sin, mask, start_pos=0, **_):
    import concourse.bass_utils as bass_utils

    x = np.asarray(x, dtype=np.float32)
    wq = np.asarray(wq, dtype=np.float32)
    wk = np.asarray(wk, dtype=np.float32)
    wv = np.asarray(wv, dtype=np.float32)
    wo = np.asarray(wo, dtype=np.float32)
    freqs_cos = np.asarray(freqs_cos, dtype=np.float32)
    freqs_sin = np.asarray(freqs_sin, dtype=np.float32)
    mask = np.asarray(mask, dtype=np.float32)

    trace = bool(int(os.environ.get("BASS_KERNEL_TRACE", "0")))
    if trace:
        _install_trace_shim()
        import concourse.bass_utils as bu
        bu.upload_artifacts = lambda tmpdir: "(upload skipped)"

    if "nc" not in _CACHE:
        _CACHE["nc"] = _build_program()
    nc = _CACHE["nc"]

    in_maps = _host_prep(x, wq, wk, wv, wo, freqs_cos, freqs_sin, mask)
    kwargs = {}
    if trace:
        kwargs = dict(trace=True, trace_cores=[0],
                      tmpdir=os.environ.get("BASS_KERNEL_TRACE_DIR", None))
    res = None
    last_exc = None
    for attempt in range(5):
        try:
            res = bass_utils.run_bass_kernel_spmd(
                nc, in_maps, core_ids=list(range(N_CORES)), **kwargs)
            break
        except Exception as e:  # transient NRT device errors recover on retry
            last_exc = e
            import time as _time
            _time.sleep(12)
    if res is None:
        raise last_exc
    _CACHE["last_result"] = res

    out = np.zeros((B, S, D), dtype=np.float32)
    for c in range(N_CORES):
        out[c // NHG] += res.results[c]["out"].astype(np.float32)
    return out
